# revision 5
# baseline (speedup 1.0000x reference)
"""Trainium2 Bass kernel for Exercise-KC GraphConvolution (concat=True branch).

Computes: elu((adj @ (kc_h @ W1)) * (ex_h @ W1 @ W2))   -> [50000, 512]

Strategy (8 NeuronCores):
  - Shard exercise rows across cores: pad 50000 -> 50176 = 8 * 49 * 128.
  - Batch-independent weight folding on host: kcWh = kc_h @ W1 ([2048, 512])
    and W12 = W1 @ W2 ([512, 512]) are precomputed in fp32 and shipped to the
    device, removing all setup matmuls from the kernel.
  - Everything streams in fp16 (measured end-to-end rel err ~4e-4 vs the 2e-2
    gate): half the HBM traffic of fp32/fp32r at the same PE rate
    (1 col/cycle), and FWL (fast weight load) fully hides LDWEIGHTS.
  - Adjacency + exercise rows fused into one k-major stream: per 128-row tile
    one 640KB DMA ([128 part, 20 k-chunks, 128 rows]) on the sync ring;
    weights/kcWh on the scalar ring; fp16 output stores on the scalar ring.
  - Per 128-row output tile: 4 accumulating matmuls for the exercise branch
    (K=512) then 16 for the spmm (K=2048), all N=512, then
    elu(x) = max(x, exp(min(x,0)) - 1) on vector+scalar engines.
"""

import numpy as np

import concourse.bass as bass
import concourse.mybir as mybir
import concourse.tile as tile
from concourse import bacc
from concourse.bass_utils import run_bass_kernel_spmd

N_EX = 50000
IN_F = 512
OUT_F = 512
N_KC = 2048
N_CORES = 8

P = 128                       # partitions
T = 49                        # row-tiles per core
E_PER_CORE = T * P            # 6272
E_PAD = N_CORES * E_PER_CORE  # 50176
KHI_ADJ = N_KC // P           # 16
KHI_IN = IN_F // P            # 4
FB = OUT_F                    # 512 (psum free dim)
KHI_TOT = KHI_ADJ + KHI_IN    # 20 combined k-chunks per row-tile

F32 = mybir.dt.float32
F16 = mybir.dt.float16


def build_nc(n_tiles: int = T):
    """Build + compile the per-core Bass program (same program on all cores)."""
    nc = bacc.Bacc(
        "TRN2",
        target_bir_lowering=False,
        debug=False,
        enable_asserts=False,
        num_devices=N_CORES,
    )
    AF = mybir.ActivationFunctionType
    OP = mybir.AluOpType

    # combined stream: 16 adj k-chunks + 4 exercise k-chunks per row-tile
    comb = nc.dram_tensor("comb", [n_tiles, P, KHI_TOT, P], F16,
                          kind="ExternalInput")
    # kcWh = kc_h @ W1 (host-folded), k-major chunks [ki][k_lo][n]
    kcw = nc.dram_tensor("kcw", [KHI_ADJ, P, FB], F16, kind="ExternalInput")
    # W12 = W1 @ W2 (host-folded), k-major [k_lo][kj][n]
    w12 = nc.dram_tensor("w12", [P, KHI_IN, FB], F16, kind="ExternalInput")
    outp = nc.dram_tensor("outp", [n_tiles, P, FB], F16, kind="ExternalOutput")

    with tile.TileContext(nc) as tc:
        with (
            tc.tile_pool(name="const", bufs=1) as constp,
            tc.tile_pool(name="adj", bufs=4) as adjp,
            tc.tile_pool(name="outb", bufs=3) as outbp,
            tc.tile_pool(name="tmp", bufs=3) as tmpp,
            tc.tile_pool(name="ps", bufs=2, space=bass.MemorySpace.PSUM) as psp,
            tc.tile_pool(name="psq", bufs=2, space=bass.MemorySpace.PSUM) as psq,
        ):
            # PE warm-up: the HAM clock gate needs ~3.4us of activity to lift
            # the 1.2GHz cold throttle; burn it on a zero tile while the
            # first DMAs are still in flight.
            warm_sb = constp.tile([P, P], F16)
            nc.vector.memset(warm_sb[:], 0.0)
            for _ in range(24):
                pwu = psq.tile([P, P], F32, tag="pw")
                nc.tensor.matmul(pwu[:], warm_sb[:], warm_sb[:],
                                 start=True, stop=True)

            # Startup DMA schedule: first two comb tiles + w12 lead on the two
            # rings, kcw chunks split across both right behind them, so the
            # PE reaches steady state as early as the ring rate allows.
            a_pre0 = adjp.tile([P, KHI_TOT, P], F16)
            nc.sync.dma_start(a_pre0[:], comb[0])
            w12_sb = constp.tile([P, KHI_IN, FB], F16)
            nc.scalar.dma_start(w12_sb[:], w12[:])
            a_pre1 = adjp.tile([P, KHI_TOT, P], F16)
            nc.scalar.dma_start(a_pre1[:], comb[1])
            kcw_sbs = []
            for ki in range(KHI_ADJ):
                kg = constp.tile([P, FB], F16, tag=f"kcw{ki}")
                eng = nc.sync if ki % 2 == 0 else nc.scalar
                eng.dma_start(kg[:], kcw[ki])
                kcw_sbs.append(kg)

            # ---- main loop over row-tiles ----
            for t in range(n_tiles):
                if t == 0:
                    a_sb = a_pre0
                elif t == 1:
                    a_sb = a_pre1
                else:
                    a_sb = adjp.tile([P, KHI_TOT, P], F16)
                    # alternate rings so load balances and startup overlaps
                    eng = nc.sync if t % 2 == 0 else nc.scalar
                    eng.dma_start(a_sb[:], comb[t])

                # exercise branch first: its PSUM->SBUF copy then overlaps
                # the 16 spmm matmuls instead of serializing after them
                ps_e = psp.tile([P, FB], F32)
                for kj in range(KHI_IN):
                    nc.tensor.matmul(
                        ps_e[:],
                        a_sb[:, KHI_ADJ + kj, :],
                        w12_sb[:, kj, :],
                        start=(kj == 0),
                        stop=(kj == KHI_IN - 1),
                    )
                exb = tmpp.tile([P, FB], F32)
                nc.scalar.copy(exb[:], ps_e[:])  # one PSUM operand max

                if t < n_tiles - 1:
                    ps_s = psp.tile([P, FB], F32)  # spmm branch
                    for ki in range(KHI_ADJ):
                        nc.tensor.matmul(
                            ps_s[:],
                            a_sb[:, ki, :],
                            kcw_sbs[ki][:],
                            start=(ki == 0),
                            stop=(ki == KHI_ADJ - 1),
                        )

                    # elu(prod) = max(prod, exp(min(prod,0)) - 1)
                    prod = tmpp.tile([P, FB], F32)
                    nc.vector.tensor_tensor(prod[:], ps_s[:], exb[:], OP.mult)
                    nmin = tmpp.tile([P, FB], F32)
                    nc.vector.tensor_scalar(nmin[:], prod[:], 0.0, None,
                                            OP.min)
                    expv = tmpp.tile([P, FB], F32)
                    nc.scalar.activation(expv[:], nmin[:], AF.Exp)
                    o_sb = outbp.tile([P, FB], F16)
                    nc.vector.scalar_tensor_tensor(
                        o_sb[:], expv[:], -1.0, prod[:], OP.add, OP.max)
                    eng = nc.scalar if t % 2 == 0 else nc.sync
                    eng.dma_start(outp[t], o_sb[:])
                else:
                    # last tile: split N in half so the elementwise chain and
                    # store of half 0 overlap the spmm matmuls of half 1,
                    # shrinking the end-of-kernel tail
                    HF = FB // 2
                    for h in range(2):
                        ps_h = psp.tile([P, HF], F32, tag="pslast")
                        for ki in range(KHI_ADJ):
                            nc.tensor.matmul(
                                ps_h[:],
                                a_sb[:, ki, :],
                                kcw_sbs[ki][:, h * HF:(h + 1) * HF],
                                start=(ki == 0),
                                stop=(ki == KHI_ADJ - 1),
                            )
                        prod = tmpp.tile([P, HF], F32, tag="prodlast")
                        nc.vector.tensor_tensor(
                            prod[:], ps_h[:], exb[:, h * HF:(h + 1) * HF],
                            OP.mult)
                        nmin = tmpp.tile([P, HF], F32, tag="nminlast")
                        nc.vector.tensor_scalar(nmin[:], prod[:], 0.0, None,
                                                OP.min)
                        expv = tmpp.tile([P, HF], F32, tag="expvlast")
                        nc.scalar.activation(expv[:], nmin[:], AF.Exp)
                        o_sb = outbp.tile([P, HF], F16, tag="olast")
                        nc.vector.scalar_tensor_tensor(
                            o_sb[:], expv[:], -1.0, prod[:], OP.add, OP.max)
                        eng = nc.scalar if h == 0 else nc.sync
                        eng.dma_start(outp[t, :, h * HF:(h + 1) * HF],
                                      o_sb[:])

    nc.compile()
    return nc


def prep_inputs(exercise_h, kc_h, adj_exercise_kc, W1, W2,
                n_tiles: int = T):
    """Host-side shard + layout prep. Returns in_maps (one dict per core)."""
    ex = np.asarray(exercise_h, dtype=np.float32)
    kc = np.asarray(kc_h, dtype=np.float32)
    adj = np.asarray(adj_exercise_kc, dtype=np.float32)
    w1 = np.asarray(W1, dtype=np.float32)
    w2 = np.asarray(W2, dtype=np.float32)

    # batch-independent weight folding (exact fp32, then one fp16 rounding)
    kcwh = (kc @ w1).astype(np.float16)                    # [2048, 512]
    w12 = (w1 @ w2).astype(np.float16)                     # [512, 512]

    e_pad = N_CORES * n_tiles * P
    n_rows = min(N_EX, e_pad)

    adj_p = np.zeros((e_pad, N_KC), np.float16)
    adj_p[:n_rows] = adj[:n_rows]
    ex_p = np.zeros((e_pad, IN_F), np.float16)
    ex_p[:n_rows] = ex[:n_rows]
    # combined [core, t, k_lo, k_hi_tot, m]: adj chunks then ex chunks
    comb = np.empty((N_CORES, n_tiles, P, KHI_TOT, P), np.float16)
    comb[:, :, :, :KHI_ADJ, :] = (
        adj_p.reshape(N_CORES, n_tiles, P, KHI_ADJ, P)
        .transpose(0, 1, 4, 3, 2))
    comb[:, :, :, KHI_ADJ:, :] = (
        ex_p.reshape(N_CORES, n_tiles, P, KHI_IN, P)
        .transpose(0, 1, 4, 3, 2))

    kcw = np.ascontiguousarray(kcwh.reshape(KHI_ADJ, P, FB))
    w12r = np.ascontiguousarray(w12.reshape(KHI_IN, P, FB).transpose(1, 0, 2))

    return [
        {"comb": comb[c], "kcw": kcw, "w12": w12r}
        for c in range(N_CORES)
    ]


def unpack_output(results, n_tiles: int = T) -> np.ndarray:
    """results: list per core of {"outp": [n_tiles, P, FB]} -> [N_EX, FB]."""
    per_core = [
        np.asarray(r["outp"]).reshape(n_tiles * P, FB)
        for r in results
    ]
    return np.concatenate(per_core, axis=0)[:N_EX].astype(np.float32)


_NC_CACHE: dict = {}


def _get_nc():
    if T not in _NC_CACHE:
        _NC_CACHE[T] = build_nc()
    return _NC_CACHE[T]


def kernel(exercise_h, kc_h, adj_exercise_kc, W1, W2):
    nc = _get_nc()
    in_maps = prep_inputs(exercise_h, kc_h, adj_exercise_kc, W1, W2)
    res = run_bass_kernel_spmd(nc, in_maps, core_ids=list(range(N_CORES)))
    return np.ascontiguousarray(unpack_output(res.results))
